# revision 1
# baseline (speedup 1.0000x reference)
"""Trainium2 Bass kernel: causal attention (QKV projection + causal softmax + AV).

Problem: x[4, 4096, 768] fp32, per-head projections to d=64, full causal
attention per batch, output [4, 4096, 64] fp32.

Sharding: 8 cores = 4 batches x 2 parity groups. Core (b, j) computes the
output rows of batch b whose 128-row block index i satisfies i % 2 == j.
One uniform SPMD program: for j=0 cores the host shifts x down by one
128-row block (prepending zeros), which makes the causal structure of both
parities identical in device coordinates (device q-blocks are always the odd
blocks 1,3,...,31; k-slot g holds true block g-1 for j=0 and g for j=1; the
dead slot 0 of j=0 is zeroed post-exp with a per-core 0/1 scale).

Device pipeline per core (all matmuls bf16, fp32 PSUM accumulation):
  P1 (per 512-row seq chunk): one 3D-output DMA-transpose yields x^T for the
     chunk; two matmul passes with stationary [wq|wq] and [wk|wv] produce
     Q^T (own q-blocks, both partition halves), K^T (low half, SWDGE-
     duplicated to the high half) and V^T (DMA-transposed into V' = [V | 1]).
  P2 (per 512-col q chunk): for consecutive k-slot pairs, two concurrent
     row-tiled matmuls K^T_g.T @ Q^T produce S^T; exp on ACT (scale 1/8,
     causal diagonal masked by a bf16 upper-tri mask, merged across the
     pair's two PSUM banks); AV accumulates V'.T @ P^T into a [65, 512]
     PSUM tile whose row 64 is the softmax denominator (ones column of V').
     The unnormalized [65, 512] tiles go to DRAM; the host divides and
     transposes.
"""

import numpy as np
import ml_dtypes
from contextlib import ExitStack

import concourse.bass as bass
import concourse.mybir as mybir
import concourse.tile as tile
from concourse import bacc
from concourse.bass_utils import run_bass_kernel_spmd

F32 = mybir.dt.float32
BF16 = mybir.dt.bfloat16

SEQ = 4096
DIN = 768
DOUT = 64
NCC = DIN // 128          # 6 contraction chunks
NSC = SEQ // 512          # 8 seq chunks (projection granularity)
NBLK = SEQ // 128         # 32 k-slots
NQC = 4                   # q chunks of 512 local columns (2048 own q rows)
SCALE = 1.0 / 8.0
EXPF = mybir.ActivationFunctionType.Exp

_CACHED_NC = None


def build_nc(dump=False, repeats=1):
    nc = bacc.Bacc("TRN2", target_bir_lowering=False, debug=False)

    x = nc.dram_tensor("x", [SEQ, DIN], BF16, kind="ExternalInput")
    wqq = nc.dram_tensor("wqq", [DIN, 128], BF16, kind="ExternalInput")  # [wq|wq]
    wkv = nc.dram_tensor("wkv", [DIN, 128], BF16, kind="ExternalInput")  # [wv|wk]
    bqq = nc.dram_tensor("bqq", [128, 1], F32, kind="ExternalInput")     # [bq;bq]
    bkv = nc.dram_tensor("bkv", [128, 1], F32, kind="ExternalInput")     # [bk;bv]
    pads = nc.dram_tensor("pads", [128, 1], F32, kind="ExternalInput")   # 1 / 0
    maska = nc.dram_tensor("maska", [128, 512], BF16, kind="ExternalInput")
    idnb = nc.dram_tensor("idnb", [64, 64], BF16, kind="ExternalInput")
    o = nc.dram_tensor("o", [NQC, 65, 512], F32, kind="ExternalOutput")
    if dump:
        okt = nc.dram_tensor("okt", [64, NBLK * 128], BF16, kind="ExternalOutput")
        oqt = nc.dram_tensor("oqt", [128, 16 * 128], BF16, kind="ExternalOutput")
        ovs = nc.dram_tensor("ovs", [128, NBLK * 65], BF16, kind="ExternalOutput")
        opt = nc.dram_tensor("opt", [128, 1024], BF16, kind="ExternalOutput")

    with tile.TileContext(nc) as tc, ExitStack() as ctx:
        cpool = ctx.enter_context(tc.tile_pool(name="const", bufs=1))
        vtp = ctx.enter_context(tc.tile_pool(name="vt", bufs=2))
        ptp = ctx.enter_context(tc.tile_pool(name="pt", bufs=3))
        ocp = ctx.enter_context(tc.tile_pool(name="oc", bufs=2))
        psproj = ctx.enter_context(tc.tile_pool(name="psproj", bufs=2, space="PSUM"))
        psst = ctx.enter_context(tc.tile_pool(name="psst", bufs=2, space="PSUM"))
        psav = ctx.enter_context(tc.tile_pool(name="psav", bufs=2, space="PSUM"))

        wqq_sb = cpool.tile([128, NCC * 128], BF16)
        wkv_sb = cpool.tile([128, NCC * 128], BF16)
        bqq_sb = cpool.tile([128, 1], F32)
        bkv_sb = cpool.tile([128, 1], F32)
        pads_sb = cpool.tile([128, 1], F32)
        mask_sb = cpool.tile([128, 512], BF16)
        idn_sb = cpool.tile([64, 64], BF16)
        kt2 = cpool.tile([128, NBLK * 128], BF16)   # K^T, both partition halves
        xtf = cpool.tile([128, NSC * NCC * 512], BF16)  # x^T, whole sequence
        qt = cpool.tile([128, 16 * 128], BF16)      # Q^T own blocks, both halves
        vs = cpool.tile([128, NBLK * 65], BF16)     # V' = [V | 1] per k-slot

        # weights laid out [c-in-chunk partition, (chunk, out_col) free]
        nc.sync.dma_start(
            wqq_sb[:].rearrange("p (cc m) -> p cc m", cc=NCC),
            wqq.rearrange("(cc p) m -> p cc m", p=128),
        )
        nc.sync.dma_start(
            wkv_sb[:].rearrange("p (cc m) -> p cc m", cc=NCC),
            wkv.rearrange("(cc p) m -> p cc m", p=128),
        )
        nc.sync.dma_start(bqq_sb[:], bqq[:, :])
        nc.sync.dma_start(bkv_sb[:], bkv[:, :])
        nc.sync.dma_start(pads_sb[:], pads[:, :])
        nc.sync.dma_start(mask_sb[:], maska[:, :])
        nc.sync.dma_start(idn_sb[:], idnb[:, :])
        # ones column of V'
        nc.vector.memset(
            vs[:].rearrange("p (g e) -> p g e", g=NBLK)[:, :, 64:65], 1.0
        )

        def xts(sc, cc):
            base = sc * NCC * 512 + cc * 512
            return xtf[:, base:base + 512]

        def trans_chunk(sc):
            """DMA-transpose x rows [sc*512, (sc+1)*512) into resident x^T."""
            nc.sync.dma_start_transpose(
                xtf[:, sc * NCC * 512:(sc + 1) * NCC * 512]
                .rearrange("p (cc s) -> p cc s", cc=NCC),
                x[sc * 512:(sc + 1) * 512, :],
            )

        def passA_chunk(sc):
            """Q^T for own (odd) q-blocks of this chunk, [wq|wq] stationary."""
            qp = psproj.tile([128, 256], F32, tag="proj")
            for cc in range(NCC):
                rhs = (
                    xts(sc, cc)
                    .rearrange("p (a b s) -> p a b s", a=2, b=2)[:, :, 1, :]
                )
                nc.tensor.matmul(
                    qp[:], wqq_sb[:, cc * 128:(cc + 1) * 128], rhs,
                    start=(cc == 0), stop=(cc == NCC - 1),
                )
            nc.vector.tensor_scalar_add(
                qt[:, sc * 256:(sc + 1) * 256], qp[:], bqq_sb[:]
            )

        def passB_chunk(sc):
            """K^T (rows 64-127) and V^T (rows 0-63), [wv|wk] stationary."""
            kp = psproj.tile([128, 512], F32, tag="proj")
            for cc in range(NCC):
                nc.tensor.matmul(
                    kp[:], wkv_sb[:, cc * 128:(cc + 1) * 128],
                    xts(sc, cc),
                    start=(cc == 0), stop=(cc == NCC - 1),
                )
            nc.vector.tensor_scalar_add(
                kt2[64:128, sc * 512:(sc + 1) * 512], kp[64:128, :], bkv_sb[64:128, :]
            )
            hi = kt2[64:128, sc * 512:(sc + 1) * 512].rearrange(
                "p (a b s) -> p a b s", a=2, b=2)[:, :, 0, :]
            lo = kt2[0:64, sc * 512:(sc + 1) * 512].rearrange(
                "p (a b s) -> p a b s", a=2, b=2)[:, :, 0, :]
            nc.gpsimd.dma_start(lo, hi)
            vt = vtp.tile([128, 512], BF16)
            nc.vector.tensor_scalar_add(
                vt[0:64, :], kp[0:64, :], bkv_sb[0:64, :]
            )
            # V' blocks via PE transpose (DMA-transpose is only HW-exact for
            # the whole-row DRAM-sourced x case)
            vp = psproj.tile([128, 256], BF16, tag="proj")
            for t in range(4):
                nc.tensor.transpose(
                    vp[:, t * 64:(t + 1) * 64],
                    vt[0:64, t * 128:(t + 1) * 128],
                    idn_sb[:],
                )
            nc.vector.tensor_copy(
                vs[:].rearrange("p (g e) -> p g e", g=NBLK)[
                    :, sc * 4:(sc + 1) * 4, 0:64
                ],
                vp[:].rearrange("p (g e) -> p g e", g=4),
            )

        parts = {}

        def attn_seg(c, p_lo, p_hi, final):
            """Attention pairs [p_lo, p_hi) for local q cols [c*512, (c+1)*512)."""
            npairs = 4 * c + 4           # k-slots 0..8c+7 in consecutive pairs
            av = psav.tile([65, 512], F32, tag="av")
            first_av = [True]

            def slot_geom(g):
                s = g - (8 * c + 1)
                if s < 1:
                    return 0, 512
                off = 128 * ((s + 1) // 2)
                return off, 512 - off

            for p in range(p_lo, p_hi):
                g0, g1 = 2 * p, 2 * p + 1
                off0, w0 = slot_geom(g0)
                off1, w1 = slot_geom(g1)
                st = psst.tile([128, 1024], F32, tag="st")
                nc.tensor.matmul(
                    st[:, 0:w0], kt2[0:64, g0 * 128:(g0 + 1) * 128],
                    qt[0:64, c * 512 + off0: c * 512 + off0 + w0],
                    start=True, stop=True, tile_position=(0, 0),
                )
                nc.tensor.matmul(
                    st[:, 512:512 + w1], kt2[64:128, g1 * 128:(g1 + 1) * 128],
                    qt[64:128, c * 512 + off1: c * 512 + off1 + w1],
                    start=True, stop=True, tile_position=(64, 0),
                )
                pt = ptp.tile([128, 1024], BF16)
                if w0 < 512:
                    # exact-width exps: skip the gap instead of memset+merge
                    nc.scalar.activation(pt[:, 0:w0], st[:, 0:w0],
                                         EXPF, bias=0.0, scale=SCALE)
                    nc.scalar.activation(pt[:, 512:512 + w1], st[:, 512:512 + w1],
                                         EXPF, bias=0.0, scale=SCALE)
                else:
                    nc.scalar.activation(pt[:, 0:512 + w1], st[:, 0:512 + w1],
                                         EXPF, bias=0.0, scale=SCALE)
                if p == 0:
                    # kill the j=0 dead slot 0 (pads = 0 there, 1 for j=1)
                    nc.vector.tensor_scalar_mul(
                        pt[:, 0:512], pt[:, 0:512], pads_sb[:]
                    )
                if p >= npairs - 4:
                    # odd member of the last four pairs is causal-diagonal
                    nc.vector.tensor_mul(
                        pt[:, 512:512 + w1], pt[:, 512:512 + w1], mask_sb[:, 0:w1]
                    )
                if dump and c == 0 and p == 0:
                    nc.sync.dma_start(opt[:, :], pt[:])
                nc.tensor.matmul(
                    av[:, off0:off0 + w0], vs[:, g0 * 65:(g0 + 1) * 65],
                    pt[:, 0:w0],
                    start=first_av[0], stop=False,
                )
                first_av[0] = False
                nc.tensor.matmul(
                    av[:, off1:off1 + w1], vs[:, g1 * 65:(g1 + 1) * 65],
                    pt[:, 512:512 + w1],
                    start=False, stop=(p == p_hi - 1),
                )
            if final:
                oc = ocp.tile([65, 512], F32)
                if c in parts:
                    nc.vector.tensor_add(oc[:], av[:], parts.pop(c)[:])
                else:
                    nc.vector.tensor_copy(oc[:], av[:])
                nc.gpsimd.dma_start(o[c, :, :], oc[:])
            else:
                part = ocp.tile([65, 512], F32, tag="part")
                nc.vector.tensor_copy(part[:], av[:])
                parts[c] = part

        # chunk 3's first attention half only needs k-slots 0-15 plus its own
        # Q columns (sc6/7): with x^T fully resident, project that Q early so
        # the exp load isn't all at the tail
        for _rep in range(repeats):
            for sc in range(NSC):
                trans_chunk(sc)
            passA_chunk(0)
            passB_chunk(0)
            passA_chunk(1)
            passB_chunk(1)
            attn_seg(0, 0, 4, True)
            passA_chunk(2)
            passB_chunk(2)
            passA_chunk(3)
            passB_chunk(3)
            attn_seg(1, 0, 8, True)
            passA_chunk(6)
            passA_chunk(7)
            attn_seg(3, 0, 8, False)
            passA_chunk(4)
            passB_chunk(4)
            passA_chunk(5)
            passB_chunk(5)
            attn_seg(2, 0, 12, True)
            passB_chunk(6)
            passB_chunk(7)
            attn_seg(3, 8, 16, True)
        if dump:
            nc.sync.dma_start(okt[:, :], kt2[64:128, :])
            nc.sync.dma_start(oqt[:, :], qt[:])
            nc.sync.dma_start(ovs[:, :], vs[:])

    nc.compile()
    return nc


def _get_nc():
    global _CACHED_NC
    if _CACHED_NC is None:
        _CACHED_NC = build_nc()
    return _CACHED_NC


def _host_inputs(x, wq, bq, wk, bk, wv, bv):
    bf = ml_dtypes.bfloat16
    wqq = np.concatenate([wq, wq], axis=1).astype(bf)
    wkv = np.concatenate([wv, wk], axis=1).astype(bf)
    bqq = np.concatenate([bq, bq])[:, None].astype(np.float32)
    bkv = np.concatenate([bv, bk])[:, None].astype(np.float32)
    tri = np.triu(np.ones((128, 128), np.float32))
    maska = np.concatenate([tri, np.ones((128, 384), np.float32)], axis=1).astype(bf)
    idnb = np.eye(64, dtype=np.float32).astype(bf)
    xbf = np.ascontiguousarray(x).astype(bf)

    in_maps = []
    for core in range(8):
        b, j = core // 2, core % 2
        if j == 0:
            xdev = np.concatenate(
                [np.zeros((128, DIN), bf), xbf[b][: SEQ - 128]], axis=0
            )
            ps = np.zeros((128, 1), np.float32)
        else:
            xdev = xbf[b]
            ps = np.ones((128, 1), np.float32)
        in_maps.append({
            "x": np.ascontiguousarray(xdev),
            "wqq": wqq, "wkv": wkv, "bqq": bqq, "bkv": bkv,
            "pads": ps, "maska": maska, "idnb": idnb,
        })
    return in_maps


def _assemble(results):
    out = np.empty((4, SEQ, DOUT), np.float32)
    for core in range(8):
        b, j = core // 2, core % 2
        od = results[core]["o"]  # [NQC, 65, 512]
        for c in range(NQC):
            num = od[c, 0:64, :].astype(np.float64)
            den = od[c, 64, :].astype(np.float64)
            oc = (num / den).T.astype(np.float32)  # [512, 64]
            for t in range(4):
                r0 = (8 * c + 2 * t + j) * 128
                out[b, r0:r0 + 128] = oc[t * 128:(t + 1) * 128]
    return out


def kernel(x, wq, bq, wk, bk, wv, bv):
    x = np.asarray(x, dtype=np.float32)
    args = [np.asarray(a, dtype=np.float32) for a in (wq, bq, wk, bk, wv, bv)]
    nc = _get_nc()
    in_maps = _host_inputs(x, *args)
    br = run_bass_kernel_spmd(nc, in_maps, core_ids=list(range(8)))
    return _assemble(br.results)

